# revision 5
# baseline (speedup 1.0000x reference)
"""ALiBi bias kernel for 8 TRN2 NeuronCores.

out[g, i, j] = -slopes[g % 16] * |i - j| for g in [0, 64), i,j in [0, 2048).

Sharding: the 64 (batch*head) slabs are split 8-per-core (core c owns slabs
8c..8c+7). The math per slab is a rank-1 expansion of a length-4095 ramp
w[k] = -slope * |k - (S-1)|: row i of the slab is w[(S-1) - i + j].

Device strategy (memory-bound, write roofline ~358 GB/s/core):
  - Host precomputes, per core, w rows for its 8 slabs (tiny: 8 x 4222 f32,
    front-padded with 127 zeros).
  - A DMA with an overlapping read AP (offset shift of -1 element per
    partition) lands tile[p, m] = w[m - p] in SBUF, so output rows
    i = a*128 + p for block a all read tile[p, (S-1 - a*128) + j] -- a
    uniform free-dim slice per block. Each 16 MiB output slab is then ONE
    SBUF->DRAM DMA with read AP [[L,128],[-128,16],[1,2048]]: pure writes,
    no per-element compute on device at all.
"""

import numpy as np

NCORES = 8
H = 16
B = 4
S = 2048
P = 128
NBLK = S // P          # 16 row-blocks per slab
L = 2 * S - 1          # 4095: length of the ramp
PAD = P - 1            # 127 zeros of front pad so reads at offset -p stay in bounds
WROW = L + PAD         # 4222
SLABS = (B * H) // NCORES  # 8 slabs per core

_COMPILED = {}


def _build_bass():
    import concourse.bass as bass
    import concourse.mybir as mybir
    from concourse.ap import AP

    nc = bass.Bass()
    w = nc.declare_dram_parameter(
        "w", [SLABS, P, L], mybir.dt.float32, isOutput=False
    )
    out = nc.declare_dram_parameter(
        "out", [SLABS, S, S], mybir.dt.float32, isOutput=True
    )

    FREE = SLABS * L  # SBUF free extent holding all 8 slab ramps

    with (
        nc.sbuf_tensor([P, FREE], mybir.dt.float32) as tile,
        nc.semaphore("load_sem") as load_sem,
        nc.semaphore("store_sem") as store_sem,
        nc.Block() as block,
    ):
        th = tile[:].tensor

        @block.scalar
        def _(scalar):
            # Per-slab loads on the ACT HWDGE ring so they overlap the SP-ring
            # stores: tile[p, s*L + m] = w[s, p, m] (host pre-shifted so that
            # tile[p, s*L + m] == ramp_s[m - p]).
            for s in range(SLABS):
                scalar.dma_start(
                    out=tile[:, s * L : (s + 1) * L], in_=w[s]
                ).then_inc(load_sem, 16)

        @block.sync
        def _(sync):
            for s in range(SLABS):
                sync.wait_ge(load_sem, 16 * (s + 1))
                st_src = AP(th, s * L + (S - 1), [[FREE, P], [-P, NBLK], [1, S]])
                st_dst = out[s].rearrange("(a p) j -> p a j", p=P)
                sync.dma_start(out=st_dst, in_=st_src).then_inc(store_sem, 16)
            sync.wait_ge(store_sem, 16 * SLABS)

    return nc


def _get_nc():
    if "nc" not in _COMPILED:
        _COMPILED["nc"] = _build_bass()
    return _COMPILED["nc"]


def _execute(slopes, trace=False, **spmd_kwargs):
    """Run the device kernel on all 8 cores; returns (full_output, results_obj)."""
    from concourse.bass_utils import run_bass_kernel_spmd

    slopes = np.asarray(slopes, dtype=np.float32)
    assert slopes.shape == (H,)

    # Host prep: per-core pre-shifted replicated ramp tiles for its 8 slabs.
    # tile[p, m] must equal ramp_s[m - p]; with the front-padded row
    # wp = [0]*PAD + ramp_s, partition p is the window wp[PAD-p : PAD-p+L].
    ramp = np.abs(np.arange(L, dtype=np.float32) - (S - 1))  # |k - (S-1)|
    in_maps = []
    for c in range(NCORES):
        wc = np.empty((SLABS, P, L), dtype=np.float32)
        for t in range(SLABS):
            g = c * SLABS + t
            wp = np.concatenate([np.zeros(PAD, np.float32), -slopes[g % H] * ramp])
            wc[t] = np.lib.stride_tricks.sliding_window_view(wp, L)[::-1]
        in_maps.append({"w": wc})

    nc = _get_nc()
    res = run_bass_kernel_spmd(
        nc, in_maps, core_ids=list(range(NCORES)), trace=trace, **spmd_kwargs
    )
    outs = [np.asarray(r["out"]).reshape(SLABS, S, S) for r in res.results]
    return np.concatenate(outs, axis=0), res


def kernel(slopes, seq_len, batch_size):
    seq_len = int(seq_len)
    batch_size = int(batch_size)
    assert seq_len == S and batch_size == B
    out, _ = _execute(slopes)
    return out


if __name__ == "__main__":
    slopes = np.random.rand(H).astype(np.float32)
    got = kernel(slopes, S, B)
    pos = np.arange(S)
    rel = np.abs(pos[:, None] - pos[None, :]).astype(np.float32)
    exp = np.tile(-slopes[:, None, None] * rel[None], (B, 1, 1))
    err = np.abs(got - exp).max()
    print("max abs err:", err, "match:", np.array_equal(got, exp))


# revision 6
# speedup vs baseline: 1.0294x; 1.0294x over previous
"""ALiBi bias kernel for 8 TRN2 NeuronCores.

out[g, i, j] = -slopes[g % 16] * |i - j| for g in [0, 64), i,j in [0, 2048).

Sharding: the 64 (batch*head) slabs are split 8-per-core (core c owns slabs
8c..8c+7). Per slab the bias is a rank-1 expansion of a length-4095 ramp
w[k] = -slope * |k - (S-1)|: row i of the slab is w[(S-1) - i + j].

Device strategy (write-bandwidth bound; SBUF-AXI fabric ~435 GB/s/core is
the binding cap, per-SDMA-engine port rate 27.2 GB/s):
  - Host sends ONE pre-shifted ramp-magnitude tile V (128 x 4095 f32, 2 MiB)
    with V[p, m] = |m - p - (S-1)| (zero where m < p), plus the 8 negated
    slopes replicated over partitions (128 x 8).
  - DVE scales V by each slab's -slope into 8 slab tiles in SBUF (~4.3 us
    each, hidden behind stores).
  - Each output slab (16 MiB) is ONE SBUF->DRAM DMA whose read AP is the
    sliding window [[FREE,128],[-128,16],[1,2048]] at offset S-1: output
    rows i = a*128 + p read tile[p, (S-1 - a*128) + j], so the whole slab
    streams out as pure writes with no further compute.
  - Stores alternate between the two HWDGE rings (sync/SP and scalar/ACT)
    so the SDMA engines always have a queue with work at slab boundaries.
"""

import numpy as np

NCORES = 8
H = 16
B = 4
S = 2048
P = 128
NBLK = S // P          # 16 row-blocks per slab
L = 2 * S - 1          # 4095: length of the ramp
PAD = P - 1            # front pad so window reads stay in bounds
SLABS = (B * H) // NCORES  # 8 slabs per core

_COMPILED = {}


def _build_bass():
    import concourse.bass as bass
    import concourse.mybir as mybir
    from concourse.ap import AP

    nc = bass.Bass()
    v = nc.declare_dram_parameter("v", [P, L], mybir.dt.float32, isOutput=False)
    ns = nc.declare_dram_parameter("ns", [P, SLABS], mybir.dt.float32, isOutput=False)
    out = nc.declare_dram_parameter(
        "out", [SLABS, S, S], mybir.dt.float32, isOutput=True
    )

    FREE = SLABS * L  # free extent of the slab-tile SBUF tensor

    with (
        nc.sbuf_tensor([P, L], mybir.dt.float32) as vt,
        nc.sbuf_tensor([P, FREE], mybir.dt.float32) as u,
        nc.sbuf_tensor([P, SLABS], mybir.dt.float32) as nst,
        nc.semaphore("load_sem") as load_sem,
        nc.semaphore("vec_sem") as vec_sem,
        nc.semaphore("store_sem") as store_sem,
        nc.Block() as block,
    ):
        uh = u[:].tensor

        def store_slab(eng, s):
            st_src = AP(uh, s * L + (S - 1), [[FREE, P], [-P, NBLK], [1, S]])
            st_dst = out[s].rearrange("(a p) j -> p a j", p=P)
            eng.dma_start(out=st_dst, in_=st_src).then_inc(store_sem, 16)

        @block.vector
        def _(vector):
            vector.wait_ge(load_sem, 32)
            for s in range(SLABS):
                vector.tensor_scalar_mul(
                    u[:, s * L : (s + 1) * L], vt[:], nst[:, s : s + 1]
                ).then_inc(vec_sem, 1)

        @block.sync
        def _(sync):
            for s in range(0, SLABS, 2):
                sync.wait_ge(vec_sem, s + 1)
                store_slab(sync, s)
            sync.wait_ge(store_sem, 16 * SLABS)

        @block.scalar
        def _(scalar):
            scalar.dma_start(out=vt[:], in_=v[:]).then_inc(load_sem, 16)
            scalar.dma_start(out=nst[:], in_=ns[:]).then_inc(load_sem, 16)
            for s in range(1, SLABS, 2):
                scalar.wait_ge(vec_sem, s + 1)
                store_slab(scalar, s)
            scalar.wait_ge(store_sem, 16 * SLABS)

    return nc


def _get_nc():
    if "nc" not in _COMPILED:
        _COMPILED["nc"] = _build_bass()
    return _COMPILED["nc"]


def _execute(slopes, trace=False, **spmd_kwargs):
    """Run the device kernel on all 8 cores; returns (full_output, results_obj)."""
    from concourse.bass_utils import run_bass_kernel_spmd

    slopes = np.asarray(slopes, dtype=np.float32)
    assert slopes.shape == (H,)

    # Pre-shifted ramp magnitude: vt[p, m] = ramp[m - p], ramp[k] = |k-(S-1)|.
    # With the front-padded row wp = [0]*PAD + ramp, partition p is the
    # window wp[PAD-p : PAD-p+L].
    ramp = np.abs(np.arange(L, dtype=np.float32) - (S - 1))
    wp = np.concatenate([np.zeros(PAD, np.float32), ramp])
    vtile = np.lib.stride_tricks.sliding_window_view(wp, L)[::-1].copy()

    in_maps = []
    for c in range(NCORES):
        neg = np.array(
            [-slopes[(c * SLABS + t) % H] for t in range(SLABS)], dtype=np.float32
        )
        in_maps.append(
            {"v": vtile, "ns": np.broadcast_to(neg, (P, SLABS)).copy()}
        )

    nc = _get_nc()
    res = run_bass_kernel_spmd(
        nc, in_maps, core_ids=list(range(NCORES)), trace=trace, **spmd_kwargs
    )
    outs = [np.asarray(r["out"]).reshape(SLABS, S, S) for r in res.results]
    return np.concatenate(outs, axis=0), res


def kernel(slopes, seq_len, batch_size):
    seq_len = int(seq_len)
    batch_size = int(batch_size)
    assert seq_len == S and batch_size == B
    out, _ = _execute(slopes)
    return out


if __name__ == "__main__":
    slopes = np.random.rand(H).astype(np.float32)
    got = kernel(slopes, S, B)
    pos = np.arange(S)
    rel = np.abs(pos[:, None] - pos[None, :]).astype(np.float32)
    exp = np.tile(-slopes[:, None, None] * rel[None], (B, 1, 1))
    err = np.abs(got - exp).max()
    print("max abs err:", err, "match:", np.array_equal(got, exp))


# revision 8
# speedup vs baseline: 1.3049x; 1.2677x over previous
"""v7: materialized quarter-slabs -> 32 KiB store descriptors.

Instead of holding one shifted ramp per partition and letting the store DMA
expand it via 2048 overlapping 8 KiB descriptors per slab, DVE materializes
4 consecutive output rows per partition (a quarter-slab = 128 parts x 4 rows
x 2048) into rotating SBUF buffers. Partition p's 4 rows are CONTIGUOUS in
the output, so each store descriptor is 32 KiB: 128 descriptors per quarter,
8 per SDMA engine, port-aligned. Per-descriptor overhead drops 4x and the
descriptor count per engine falls from 1024 to 256 per run.

V2[p, k] = ramp[k - 4p]; quarter q of slab s covers rows i = 512q + 4p + t,
value = -slope_s * ramp[(S-1) - i + j] = nst[s] * V2[p, (S-1 - 512q) - t + j]
-- a uniform (t, j) window AP across partitions, one tensor_scalar per
quarter (~4.3 us at DVE 2x) hidden under the ~9.7 us store.
"""

import numpy as np

NCORES = 8
H = 16
B = 4
S = 2048
P = 128
L = 2 * S - 1
SLABS = (B * H) // NCORES

RPP = 4                    # rows per partition per chunk
QROWS = P * RPP            # 512 rows per chunk
NQ = S // QROWS            # 4 chunks per slab
CHUNK_FREE = RPP * S       # 8192 elems per partition per chunk
NBUF = 4

_COMPILED = {}


def _build_bass():
    import concourse.bass as bass
    import concourse.mybir as mybir
    from concourse.ap import AP

    nc = bass.Bass()
    v = nc.declare_dram_parameter("v", [P, L], mybir.dt.float32, isOutput=False)
    ns = nc.declare_dram_parameter("ns", [P, SLABS], mybir.dt.float32, isOutput=False)
    out = nc.declare_dram_parameter(
        "out", [SLABS, S, S], mybir.dt.float32, isOutput=True
    )

    with (
        nc.sbuf_tensor([P, L], mybir.dt.float32) as vt,
        nc.sbuf_tensor([P, NBUF * CHUNK_FREE], mybir.dt.float32) as u,
        nc.sbuf_tensor([P, SLABS], mybir.dt.float32) as nst,
        nc.semaphore("load_sem") as load_sem,
        nc.semaphore("vec_sem") as vec_sem,
        nc.semaphore("st0") as st0,
        nc.semaphore("st1") as st1,
        nc.semaphore("st2") as st2,
        nc.semaphore("st3") as st3,
        nc.Block() as block,
    ):
        slot_sems = [st0, st1, st2, st3]
        uh = u[:].tensor
        vh = vt[:].tensor
        UFREE = NBUF * CHUNK_FREE

        # chunk q reads V2[:, (S-1) - 512q - t + j]: chunk 0 needs only
        # k >= S-1-RPP+1; load that tail first so DVE starts ~3 us earlier.
        VSPLIT = S - RPP

        @block.scalar
        def _(scalar):
            scalar.dma_start(out=nst[:], in_=ns[:]).then_inc(load_sem, 16)
            scalar.dma_start(
                out=vt[:, VSPLIT:L], in_=v[:, VSPLIT:L]
            ).then_inc(load_sem, 16)
            scalar.dma_start(
                out=vt[:, 0:VSPLIT], in_=v[:, 0:VSPLIT]
            ).then_inc(load_sem, 16)

        @block.vector
        def _(vector):
            for k in range(SLABS * NQ):
                if k == 0:
                    vector.wait_ge(load_sem, 32)   # ns + V2 tail
                elif k == 1:
                    vector.wait_ge(load_sem, 48)   # full V2
                s, q = divmod(k, NQ)
                if k >= NBUF:
                    # slot sem is only bumped by this slot's stores, and the
                    # (k//NBUF+1)-th such store cannot have been issued yet,
                    # so >= 16*(k//NBUF) == "previous store on this slot fully
                    # drained on all 16 engines" (no cross-store interleave).
                    vector.wait_ge(slot_sems[k % NBUF], 16 * (k // NBUF))
                buf = (k % NBUF) * CHUNK_FREE
                src = AP(vh, (S - 1) - QROWS * q, [[L, P], [-1, RPP], [1, S]])
                dst = AP(uh, buf, [[UFREE, P], [S, RPP], [1, S]])
                vector.tensor_scalar_mul(dst, src, nst[:, s : s + 1]).then_inc(
                    vec_sem, 1
                )

        @block.gpsimd
        def _(gpsimd):
            for k in range(SLABS * NQ):
                s, q = divmod(k, NQ)
                gpsimd.wait_ge(vec_sem, k + 1)
                buf = (k % NBUF) * CHUNK_FREE
                gpsimd.dma_start(
                    out=AP(
                        out,
                        (s * S + QROWS * q) * S,
                        [[RPP * S, P], [1, RPP * S]],
                    ),
                    in_=AP(uh, buf, [[UFREE, P], [1, CHUNK_FREE]]),
                ).then_inc(slot_sems[k % NBUF], 16)
            for b in range(NBUF):
                gpsimd.wait_ge(slot_sems[b], 16 * (SLABS * NQ // NBUF))

    return nc


def _get_nc():
    if "nc" not in _COMPILED:
        _COMPILED["nc"] = _build_bass()
    return _COMPILED["nc"]


def _execute(slopes, trace=False, **spmd_kwargs):
    from concourse.bass_utils import run_bass_kernel_spmd

    slopes = np.asarray(slopes, dtype=np.float32)
    assert slopes.shape == (H,)

    # V2[p, k] = ramp[k - RPP*p] (zeros where k < RPP*p)
    ramp = np.abs(np.arange(L, dtype=np.float32) - (S - 1))
    wp = np.concatenate([np.zeros(RPP * (P - 1), np.float32), ramp])
    vtile = np.lib.stride_tricks.sliding_window_view(wp, L)[:: -RPP].copy()
    assert vtile.shape == (P, L)

    in_maps = []
    for c in range(NCORES):
        neg = np.array(
            [-slopes[(c * SLABS + t) % H] for t in range(SLABS)], dtype=np.float32
        )
        in_maps.append({"v": vtile, "ns": np.broadcast_to(neg, (P, SLABS)).copy()})

    nc = _get_nc()
    res = run_bass_kernel_spmd(
        nc, in_maps, core_ids=list(range(NCORES)), trace=trace, **spmd_kwargs
    )
    outs = [np.asarray(r["out"]).reshape(SLABS, S, S) for r in res.results]
    return np.concatenate(outs, axis=0), res


def kernel(slopes, seq_len, batch_size):
    seq_len = int(seq_len)
    batch_size = int(batch_size)
    assert seq_len == S and batch_size == B
    out, _ = _execute(slopes)
    return out


# revision 9
# speedup vs baseline: 1.3092x; 1.0033x over previous
"""v8 = v7 + fast-start prologue.

Slab 0's first 512 rows are produced as four 128-row chunks from a shift-1
tile V1s (1 row per partition, 8 KiB descriptors) so the first store
descriptors flow ~10 us earlier; everything else uses the v7 structure
(shift-4 tile V2, 512-row chunks, 32 KiB descriptors, SWDGE stores,
per-buffer-slot semaphores).

Chunk list: [(row0, nrows, which-tile)] per slab; only slab 0 differs.
"""

import numpy as np

NCORES = 8
H = 16
B = 4
S = 2048
P = 128
L = 2 * S - 1
SLABS = (B * H) // NCORES

RPP = 4                    # rows per partition in a V2 chunk
QROWS = P * RPP            # 512
NQ = S // QROWS
CHUNK_FREE = RPP * S       # slot size (V1 chunks use only S of it)
NBUF = 4

# V1s covers ramp window k in [V1OFF, 4095): chunks c=0..3 read
# k = (S-1) - 128c + j in [1663, 4095)
V1OFF = (S - 1) - P * 3
V1LEN = L - V1OFF          # 2432

# per-slab chunk schedule: (row0, rpp) with rpp==1 -> V1s, rpp==4 -> V2
SLAB0 = [(P * c, 1) for c in range(4)] + [(QROWS * (q + 1), 4) for q in range(3)]
SLABN = [(QROWS * q, 4) for q in range(NQ)]

_COMPILED = {}


def _build_bass():
    import concourse.bass as bass
    import concourse.mybir as mybir
    from concourse.ap import AP

    nc = bass.Bass()
    v = nc.declare_dram_parameter("v", [P, L], mybir.dt.float32, isOutput=False)
    v1 = nc.declare_dram_parameter("v1", [P, V1LEN], mybir.dt.float32, isOutput=False)
    ns = nc.declare_dram_parameter("ns", [P, SLABS], mybir.dt.float32, isOutput=False)
    out = nc.declare_dram_parameter(
        "out", [SLABS, S, S], mybir.dt.float32, isOutput=True
    )

    chunks = []  # (slab, row0, rpp)
    for s in range(SLABS):
        for row0, rpp in (SLAB0 if s == 0 else SLABN):
            chunks.append((s, row0, rpp))
    NCHUNK = len(chunks)

    with (
        nc.sbuf_tensor([P, L], mybir.dt.float32) as vt,
        nc.sbuf_tensor([P, V1LEN], mybir.dt.float32) as v1t,
        nc.sbuf_tensor([P, NBUF * CHUNK_FREE], mybir.dt.float32) as u,
        nc.sbuf_tensor([P, SLABS], mybir.dt.float32) as nst,
        nc.semaphore("load_sem") as load_sem,
        nc.semaphore("vec_sem") as vec_sem,
        nc.semaphore("st0") as st0,
        nc.semaphore("st1") as st1,
        nc.semaphore("st2") as st2,
        nc.semaphore("st3") as st3,
        nc.Block() as block,
    ):
        slot_sems = [st0, st1, st2, st3]
        uh = u[:].tensor
        vh = vt[:].tensor
        v1h = v1t[:].tensor
        UFREE = NBUF * CHUNK_FREE

        @block.scalar
        def _(scalar):
            scalar.dma_start(out=nst[:], in_=ns[:]).then_inc(load_sem, 16)
            scalar.dma_start(out=v1t[:], in_=v1[:]).then_inc(load_sem, 16)
            scalar.dma_start(out=vt[:], in_=v[:]).then_inc(load_sem, 16)

        @block.vector
        def _(vector):
            for k, (s, row0, rpp) in enumerate(chunks):
                if k == 0:
                    vector.wait_ge(load_sem, 32)   # ns + V1s
                elif chunks[k - 1][2] == 1 and rpp == 4:
                    vector.wait_ge(load_sem, 48)   # full V2 before first V2 chunk
                if k >= NBUF:
                    vector.wait_ge(slot_sems[k % NBUF], 16 * (k // NBUF))
                buf = (k % NBUF) * CHUNK_FREE
                if rpp == 1:
                    src = AP(v1h, (S - 1) - row0 - V1OFF, [[V1LEN, P], [1, S]])
                    dst = AP(uh, buf, [[UFREE, P], [1, S]])
                else:
                    src = AP(vh, (S - 1) - row0, [[L, P], [-1, RPP], [1, S]])
                    dst = AP(uh, buf, [[UFREE, P], [S, RPP], [1, S]])
                vector.tensor_scalar_mul(dst, src, nst[:, s : s + 1]).then_inc(
                    vec_sem, 1
                )

        @block.gpsimd
        def _(gpsimd):
            slot_counts = [0] * NBUF
            for k, (s, row0, rpp) in enumerate(chunks):
                gpsimd.wait_ge(vec_sem, k + 1)
                buf = (k % NBUF) * CHUNK_FREE
                nel = rpp * S
                gpsimd.dma_start(
                    out=AP(out, (s * S + row0) * S, [[nel, P], [1, nel]]),
                    in_=AP(uh, buf, [[UFREE, P], [1, nel]]),
                ).then_inc(slot_sems[k % NBUF], 16)
                slot_counts[k % NBUF] += 1
            for b in range(NBUF):
                gpsimd.wait_ge(slot_sems[b], 16 * slot_counts[b])

    return nc


def _get_nc():
    if "nc" not in _COMPILED:
        _COMPILED["nc"] = _build_bass()
    return _COMPILED["nc"]


def _execute(slopes, trace=False, **spmd_kwargs):
    from concourse.bass_utils import run_bass_kernel_spmd

    slopes = np.asarray(slopes, dtype=np.float32)
    assert slopes.shape == (H,)

    ramp = np.abs(np.arange(L, dtype=np.float32) - (S - 1))
    # V2[p, k] = ramp[k - 4p]
    wp = np.concatenate([np.zeros(RPP * (P - 1), np.float32), ramp])
    v2tile = np.lib.stride_tricks.sliding_window_view(wp, L)[::-RPP].copy()
    # V1s[p, k'] = ramp[k' + V1OFF - p]
    wp1 = np.concatenate([np.zeros(P - 1, np.float32), ramp])
    v1full = np.lib.stride_tricks.sliding_window_view(wp1, L)[::-1]
    v1tile = v1full[:, V1OFF:].copy()

    in_maps = []
    for c in range(NCORES):
        neg = np.array(
            [-slopes[(c * SLABS + t) % H] for t in range(SLABS)], dtype=np.float32
        )
        in_maps.append(
            {
                "v": v2tile,
                "v1": v1tile,
                "ns": np.broadcast_to(neg, (P, SLABS)).copy(),
            }
        )

    nc = _get_nc()
    res = run_bass_kernel_spmd(
        nc, in_maps, core_ids=list(range(NCORES)), trace=trace, **spmd_kwargs
    )
    outs = [np.asarray(r["out"]).reshape(SLABS, S, S) for r in res.results]
    return np.concatenate(outs, axis=0), res


def kernel(slopes, seq_len, batch_size):
    seq_len = int(seq_len)
    batch_size = int(batch_size)
    assert seq_len == S and batch_size == B
    out, _ = _execute(slopes)
    return out
